# revision 1
# baseline (speedup 1.0000x reference)
"""Ragged-sequence multi-head attention (B=16, S=1024, D=512, H=8, DH=64)
for 8 Trainium2 NeuronCores.

Strategy: data-parallel over the batch. The 16 sequences are sorted by
length; the 8 longest go to slot 0 (one per core), the 8 shortest to
slot 1. A single SPMD Bass program processes both slots with per-slot
static loop bounds equal to ceil128(max length in that slot); within a
bound, invalid key positions are masked via a per-partition additive
bias on the exp() activation, and padded query rows are zeroed via a
per-partition multiplicative mask.

Per-core pipeline (per slot, all fp32 / fp32r):
  1. x -> xT (PE transpose via identity)
  2. QT = Wq^T @ x^T, KT likewise (feature-major), V in [s, d] layout
  3. per head-pair, per q-chunk, per k-tile:
       scoresT[k, q] = K^T q   (row-packed head pair on the PE array)
       expT = exp(0.125 * scoresT + key_mask_bias)   (ACT engine)
       outT[d, q]  += V^T expT (col-packed head pair)
       denom[., q] += 1^T expT (col-packed head pair, replicated rows)
  4. outT_norm = outT * reciprocal(denom)   (DVE)
  5. out[s, d] = outT_norm^T @ Wo + bo, masked by query validity
"""

import math
import os

import numpy as np

B, S, D = 16, 1024, 512
H, DH = 8, 64
N_CORES = 8
P = 128  # partitions
KC = D // P  # 4 contraction chunks of 128
NT_MAX = S // P  # 8 key tiles max

_BUILD_CACHE: dict = {}


def _ceil128(n: int) -> int:
    return max(P, (int(n) + P - 1) // P * P)


def _build_bass(bounds: tuple[int, int]):
    """Build the Bass program for per-slot bounds (multiples of 128)."""
    from contextlib import ExitStack

    import concourse.bass as bass
    import concourse.mybir as mybir
    import concourse.tile as tile
    from concourse import bacc

    fp32 = mybir.dt.float32
    fp16 = mybir.dt.float16
    Exp = mybir.ActivationFunctionType.Exp
    mult = mybir.AluOpType.mult
    add = mybir.AluOpType.add

    nc = bacc.Bacc("TRN2", target_bir_lowering=False, debug=False)

    xin = nc.dram_tensor("xin", [2, S, D], fp32, kind="ExternalInput").ap()
    ident_d = nc.dram_tensor("ident", [P, P], fp32, kind="ExternalInput").ap()
    kbias_d = nc.dram_tensor("kbias", [2, P, NT_MAX], fp32, kind="ExternalInput").ap()
    qmask_d = nc.dram_tensor("qmask", [2, P, NT_MAX], fp32, kind="ExternalInput").ap()
    w_d = {
        name: nc.dram_tensor(name, [D, D], fp32, kind="ExternalInput").ap()
        for name in ("wq", "wk", "wv", "wo")
    }
    bo_d = nc.dram_tensor("bo", [D], fp32, kind="ExternalInput").ap()
    out_d = nc.dram_tensor("out", [2, S, D], fp32, kind="ExternalOutput").ap()

    NT = [bounds[0] // P, bounds[1] // P]
    QCH = [
        [(qs, min(512, bounds[b] - qs)) for qs in range(0, bounds[b], 512)]
        for b in (0, 1)
    ]

    with ExitStack() as ctx:
        tc = ctx.enter_context(tile.TileContext(nc))
        singles = ctx.enter_context(tc.tile_pool(name="singles", bufs=1))
        wstage_p = ctx.enter_context(tc.tile_pool(name="wstage_p", bufs=2))
        big = ctx.enter_context(tc.tile_pool(name="big", bufs=1))
        xpool = ctx.enter_context(tc.tile_pool(name="xpool", bufs=4))
        epool = ctx.enter_context(tc.tile_pool(name="epool", bufs=3))
        opool = ctx.enter_context(tc.tile_pool(name="opool", bufs=4))
        mmps = ctx.enter_context(tc.tile_pool(name="mmps", bufs=2, space="PSUM"))
        scps = ctx.enter_context(tc.tile_pool(name="scps", bufs=2, space="PSUM"))
        accps = ctx.enter_context(tc.tile_pool(name="accps", bufs=1, space="PSUM"))

        # ---- weights / constants ----
        ones64 = singles.tile([P, DH], fp16)
        nc.vector.memset(ones64, 1.0)
        w_sb = {}
        for name in ("wv", "wq", "wk", "wo"):
            w_sb[name] = singles.tile(
                [P, KC, D], fp16, name=f"w_{name}", tag=f"w_{name}"
            )
        def load_weight(name):
            ws = wstage_p.tile([P, KC, D], fp32, name=f"ws_{name}", tag="wstage")
            nc.sync.dma_start(
                out=ws, in_=w_d[name].rearrange("(kc p) n -> p kc n", p=P)
            )
            nc.scalar.copy(out=w_sb[name], in_=ws)

        for name in ("wv", "wq"):
            load_weight(name)

        # ---- phase A first: x DMAs + transposes (no weights needed) ----
        identity = singles.tile([P, P], fp32)
        nc.sync.dma_start(out=identity, in_=ident_d)
        xT = []
        for b in (0, 1):
            xT.append(big.tile([P, KC, bounds[b]], fp16, name=f"xT{b}", tag=f"xT{b}"))
            for st in range(NT[b]):
                x_tile = xpool.tile([P, D], fp32, tag="x_tile")
                nc.sync.dma_start(out=x_tile, in_=xin[b, st * P : (st + 1) * P, :])
                xt_ps = mmps.tile([P, 512], fp32, name="xt_ps", tag="mm")
                for dc in range(KC):
                    nc.tensor.transpose(
                        xt_ps[:, dc * P : (dc + 1) * P],
                        x_tile[:, dc * P : (dc + 1) * P],
                        identity,
                    )
                nc.vector.tensor_copy(
                    out=xT[b][:, :, st * P : (st + 1) * P],
                    in_=xt_ps.rearrange("p (dc c) -> p dc c", dc=KC),
                )

        for name in ("wk", "wo"):
            load_weight(name)
        kbias_sb = singles.tile([P, 2, NT_MAX], fp32)
        nc.sync.dma_start(out=kbias_sb, in_=kbias_d.rearrange("b p t -> p b t"))
        qmask_sb = singles.tile([P, 2, NT_MAX], fp32)
        nc.sync.dma_start(out=qmask_sb, in_=qmask_d.rearrange("b p t -> p b t"))
        bo_rep = singles.tile([P, D], fp32)
        bo_bcast = bass.AP(tensor=bo_d.tensor, offset=bo_d.offset, ap=[[0, P], [1, D]])
        nc.gpsimd.dma_start(out=bo_rep, in_=bo_bcast)

        # ---- V: slot 0 emitted now; slot 1 rides the filler ----
        V = [
            big.tile([P, NT[b], D], fp16, name=f"V{b}", tag=f"V{b}")
            for b in (0, 1)
        ]

        def v_units(b, st):
            ps_box = []

            def mk_mm(kc):
                def emit():
                    if not ps_box:
                        ps_box.append(
                            mmps.tile([P, 512], fp32, name="v_ps", tag="mm")
                        )
                    nc.tensor.matmul(
                        ps_box[0],
                        xT[b][:, kc, st * P : (st + 1) * P],
                        w_sb["wv"][:, kc, :],
                        start=(kc == 0),
                        stop=(kc == KC - 1),
                    )
                return emit

            def fin():
                nc.vector.tensor_copy(out=V[b][:, st, :], in_=ps_box[0])

            return [mk_mm(kc) for kc in range(KC)] + [fin]

        for st in range(NT[0]):
            for u in v_units(0, st):
                u()

        QT = [
            big.tile([P, KC, bounds[b]], fp16, name=f"QT{b}", tag=f"QT{b}")
            for b in (0, 1)
        ]
        KT = [
            big.tile([P, KC, bounds[b]], fp16, name=f"KT{b}", tag=f"KT{b}")
            for b in (0, 1)
        ]
        outT = [
            big.tile([P, KC, bounds[b]], fp16, name=f"oT{b}", tag=f"oT{b}")
            for b in (0, 1)
        ]

        def qtkt_units(b, hp, dst, wname, qs, w):
            ps_box = []

            def mk_mm(kc):
                def emit():
                    if not ps_box:
                        ps_box.append(
                            mmps.tile([P, 512], fp32, name="qk_ps", tag="mm")
                        )
                    nc.tensor.matmul(
                        ps_box[0][:, :w],
                        w_sb[wname][:, kc, hp * P : (hp + 1) * P],
                        xT[b][:, kc, qs : qs + w],
                        start=(kc == 0),
                        stop=(kc == KC - 1),
                    )
                return emit

            def fin():
                nc.vector.tensor_copy(
                    out=dst[:, hp, qs : qs + w], in_=ps_box[0][:, :w]
                )

            return [mk_mm(kc) for kc in range(KC)] + [fin]

        def outproj_units(b, st):
            ps_box = []

            def mk_mm(hc):
                def emit():
                    if not ps_box:
                        ps_box.append(
                            mmps.tile([P, 512], fp32, name="fo_ps", tag="mm")
                        )
                    nc.tensor.matmul(
                        ps_box[0],
                        outT[b][:, hc, st * P : (st + 1) * P],
                        w_sb["wo"][:, hc, :],
                        start=(hc == 0),
                        stop=(hc == KC - 1),
                    )
                return emit

            def fin():
                fout = opool.tile([P, D], fp32, tag="fout")
                nc.vector.tensor_tensor(fout, ps_box[0], bo_rep, add)
                nc.vector.tensor_scalar_mul(
                    fout, fout, qmask_sb[:, b, st : st + 1]
                )
                nc.sync.dma_start(
                    out=out_d[b, st * P : (st + 1) * P, :], in_=fout
                )

            return [mk_mm(hc) for hc in range(KC)] + [fin]

        def attn_chunk(b, hp, qs, w, filler, iters_left):
            o_ps = accps.tile([P, 512], fp32, name="o_ps", tag="o_ps")
            d_ps = accps.tile([P, 512], fp32, name="d_ps", tag="d_ps")
            nt = NT[b]

            def emit_scores_exp(kt):
                s_pair = scps.tile([P, 1024], fp32, name="s_pair", tag="s_pair")
                nc.tensor.matmul(
                    s_pair[:, 0:w],
                    KT[b][0:DH, hp, kt * P : (kt + 1) * P],
                    QT[b][0:DH, hp, qs : qs + w],
                    start=True, stop=True, tile_position=(0, 0),
                )
                nc.tensor.matmul(
                    s_pair[:, 512 : 512 + w],
                    KT[b][DH:P, hp, kt * P : (kt + 1) * P],
                    QT[b][DH:P, hp, qs : qs + w],
                    start=True, stop=True, tile_position=(DH, 0),
                )
                e_pair = epool.tile([P, 2, 512], fp16, name="e_pair", tag="e_pair")
                nc.scalar.activation(
                    e_pair[:, :, :w],
                    s_pair.rearrange("p (h q) -> p h q", h=2)[:, :, :w],
                    Exp, bias=kbias_sb[:, b, kt : kt + 1], scale=DH**-0.5,
                )
                return e_pair

            def emit_pv(kt, e_pair):
                first, last = kt == 0, kt == nt - 1
                nc.tensor.matmul(
                    o_ps[0:DH, :w], V[b][:, kt, hp * P : hp * P + DH],
                    e_pair[:, 0, :w], start=first, stop=last,
                    tile_position=(0, 0), skip_group_check=True,
                )
                nc.tensor.matmul(
                    o_ps[DH:P, :w], V[b][:, kt, hp * P + DH : (hp + 1) * P],
                    e_pair[:, 1, :w], start=first, stop=last,
                    tile_position=(0, DH), skip_group_check=True,
                )
                nc.tensor.matmul(
                    d_ps[0:DH, :w], ones64, e_pair[:, 0, :w],
                    start=first, stop=last,
                    tile_position=(0, 0), skip_group_check=True,
                )
                nc.tensor.matmul(
                    d_ps[DH:P, :w], ones64, e_pair[:, 1, :w],
                    start=first, stop=last,
                    tile_position=(0, DH), skip_group_check=True,
                )

            pending = None
            for kt in range(nt):
                e_pair = emit_scores_exp(kt)
                if pending is not None:
                    emit_pv(*pending)
                pending = (kt, e_pair)
                if filler and iters_left[0] > 0:
                    k = -(-len(filler) // iters_left[0])
                    for _ in range(min(k, len(filler))):
                        filler.pop(0)()
                iters_left[0] -= 1
            emit_pv(*pending)
            rrep = epool.tile([P, 512], fp32, tag="rrep", bufs=2)
            nc.vector.reciprocal_approx_fast(out=rrep[:, :w], in_=d_ps[:, :w])
            nc.vector.tensor_tensor(
                outT[b][:, hp, qs : qs + w], o_ps[:, :w], rrep[:, :w], mult
            )

        # ---- choreographed emission ----
        for dst, wname in ((QT[0], "wq"), (KT[0], "wk")):
            for qs, w in QCH[0]:
                for u in qtkt_units(0, 0, dst, wname, qs, w):
                    u()

        blocks = [(0, hp) for hp in range(KC)] + [(1, hp) for hp in range(KC)]
        during_block = [[] for _ in blocks]
        # V for slot 1 drains during slot0 hp0/hp1
        for st in range(NT[1]):
            during_block[st % 2].extend(v_units(1, st))
        for j in range(1, len(blocks)):
            b, hp = blocks[j]
            for dst, wname in ((QT[b], "wq"), (KT[b], "wk")):
                for qs, w in QCH[b]:
                    during_block[j - 1].extend(
                        qtkt_units(b, hp, dst, wname, qs, w)
                    )
        # slot-0 output projection rides along slot-1's attention blocks
        s1_blocks = list(range(KC, 2 * KC))
        d0_units = [u for st in range(NT[0]) for u in outproj_units(0, st)]
        per_block = -(-len(d0_units) // len(s1_blocks))
        for i, j in enumerate(s1_blocks):
            during_block[j].extend(d0_units[i * per_block : (i + 1) * per_block])

        filler: list = []
        for i, (b, hp) in enumerate(blocks):
            filler.extend(during_block[i])
            iters_left = [len(QCH[b]) * NT[b]]
            for qs, w in QCH[b]:
                attn_chunk(b, hp, qs, w, filler, iters_left)
            while filler:
                filler.pop(0)()

        # slot-1 output projection (tail)
        for st in range(NT[1]):
            for u in outproj_units(1, st):
                u()

    nc.compile()
    return nc


def _get_program(bounds: tuple[int, int]):
    key = bounds
    if key not in _BUILD_CACHE:
        _BUILD_CACHE[key] = _build_bass(bounds)
    return _BUILD_CACHE[key]


def kernel(x, seq_lens, Wq, Wk, Wv, Wo, bo) -> np.ndarray:
    from concourse.bass_utils import run_bass_kernel_spmd

    x = np.ascontiguousarray(np.asarray(x, dtype=np.float32))
    seq_lens_np = np.asarray(seq_lens, dtype=np.int32)
    Wq = np.ascontiguousarray(np.asarray(Wq, dtype=np.float32))
    Wk = np.ascontiguousarray(np.asarray(Wk, dtype=np.float32))
    Wv = np.ascontiguousarray(np.asarray(Wv, dtype=np.float32))
    Wo = np.ascontiguousarray(np.asarray(Wo, dtype=np.float32))
    bo = np.ascontiguousarray(np.asarray(bo, dtype=np.float32))

    # Sort sequences by length: longest 8 -> slot 0, rest -> slot 1.
    order = np.argsort(-seq_lens_np, kind="stable")
    slot_seqs = [order[:N_CORES], order[N_CORES:]]
    bounds = tuple(int(_ceil128(seq_lens_np[s].max())) for s in slot_seqs)

    nc = _get_program(bounds)

    # Per-partition masks laid out as [slot, p, tile]: position t*128+p.
    pos = (np.arange(NT_MAX)[None, :] * P + np.arange(P)[:, None]).astype(np.int32)
    in_maps = []
    for c in range(N_CORES):
        seq_pair = [int(slot_seqs[0][c]), int(slot_seqs[1][c])]
        xin = np.stack([x[seq_pair[0]], x[seq_pair[1]]])
        kbias = np.zeros((2, P, NT_MAX), dtype=np.float32)
        qmask = np.zeros((2, P, NT_MAX), dtype=np.float32)
        for slot, seq in enumerate(seq_pair):
            valid = pos < int(seq_lens_np[seq])
            kbias[slot] = np.where(valid, 0.0, -60.0)
            qmask[slot] = valid.astype(np.float32)
        in_maps.append(
            {
                "xin": xin,
                "ident": np.eye(P, dtype=np.float32),
                "kbias": kbias,
                "qmask": qmask,
                "wq": Wq,
                "wk": Wk,
                "wv": Wv,
                "wo": Wo,
                "bo": bo,
            }
        )

    trace = bool(int(os.environ.get("KERNEL_TRACE", "0")))
    res = run_bass_kernel_spmd(
        nc, in_maps, core_ids=list(range(N_CORES)), trace=trace
    )
    kernel.last_results = res

    out = np.zeros((B, S, D), dtype=np.float32)
    for c in range(N_CORES):
        out[int(slot_seqs[0][c])] = res.results[c]["out"][0]
        out[int(slot_seqs[1][c])] = res.results[c]["out"][1]
    return out



# revision 3
# speedup vs baseline: 1.3580x; 1.3580x over previous
"""Ragged-sequence multi-head attention (B=16, S=1024, D=512, H=8, DH=64)
for 8 Trainium2 NeuronCores.

Strategy: per-core specialized Bass programs over a balanced ragged job
assignment. Each sequence is rounded up to 128-token tiles; long
sequences are split by query-tile range across cores (each split re-hosts
the full K/V of its parent sequence). The host:
  - pre-transposes x into feature-major fp16 tiles (no PE transposes),
  - pre-converts the four projection weights to fp16 in [P, KC, D] layout,
  - packs each core's sequences into a flat tile arena with per-tile
    key-validity biases and query masks,
  - compiles one Bass program per distinct core *structure* (programs are
    shared between cores whose job shapes match; data differs via inputs),
  - dispatches all 8 single-core executables asynchronously (they overlap
    on the 8 NeuronCores) and scatters the per-tile outputs back.

Per-core pipeline (all matmuls fp16 operands, fp32 PSUM):
  KT/QT = W^T @ xT (feature-major), V in [tile, d] layout, then per
  head-pair/q-chunk/k-tile: scoresT = K^T q (row-packed pair),
  expT = exp(0.125*scoresT + kbias) on ACT, outT += V^T expT and
  denom += 1^T expT (col-packed pairs), normalize by reciprocal,
  out = outT^T @ Wo + bo, query-masked, DMA out.
"""

import math
import os

import numpy as np

B, S, D = 16, 1024, 512
H, DH = 8, 64
N_CORES = 8
P = 128
KC = D // P  # 4 contraction chunks of 128

_BUILD_CACHE: dict = {}
_FN_CACHE: dict = {}


# --------------------------------------------------------------------------
# planning: split sequences into blocks, balance blocks across cores
# --------------------------------------------------------------------------

def _chunks_of(m_tiles: int) -> tuple:
    """Split m q-tiles into chunk widths (tokens), biggest first, <=512."""
    out = []
    left = m_tiles * P
    while left > 0:
        w = min(512, left)
        out.append(w)
        left -= w
    return tuple(out)


def _plan(seq_lens: np.ndarray):
    """Returns per-core list of groups (seq, nk, q_tiles list)."""
    n = [max(1, -(-int(L) // P)) for L in seq_lens]
    # blocks: (seq, nk, q_tiles)
    blocks = []
    for i, nk in enumerate(n):
        if nk >= 7:  # split big sequences in half by q-tiles
            h = nk // 2
            blocks.append((i, nk, list(range(0, nk - h))))
            blocks.append((i, nk, list(range(nk - h, nk))))
        else:
            blocks.append((i, nk, list(range(nk))))
    # LPT greedy on max(PE, ACT) estimate
    PE_UNIT, PE_TILE = 640.0, 1707.0

    def act_block(nk, m):
        t = 0.0
        for w in _chunks_of(m):
            t += nk * 4 * (2 * w + 352) / 1.2
        return t

    blocks.sort(key=lambda b: -(b[1] * len(b[2])))
    cores = [
        {"pe": 0.0, "act": 0.0, "groups": []} for _ in range(N_CORES)
    ]
    for seq, nk, qts in blocks:
        u = nk * len(qts)
        best, bestscore = None, None
        for c in cores:
            pe = c["pe"] + PE_UNIT * u + PE_TILE * (nk + len(qts))
            act = c["act"] + act_block(nk, len(qts))
            score = max(pe, act)
            if bestscore is None or score < bestscore:
                best, bestscore = c, score
        best["pe"] += PE_UNIT * u + PE_TILE * (nk + len(qts))
        best["act"] += act_block(nk, len(qts))
        best["groups"].append((seq, nk, qts))
    return [c["groups"] for c in cores]


# --------------------------------------------------------------------------
# program generator, parameterized by core structure
# --------------------------------------------------------------------------

def _spec_of(groups):
    """Structural spec: tuple of (nk, chunk widths). Data-independent."""
    return tuple((nk, _chunks_of(len(qts))) for _, nk, qts in groups)


def _build_bass(spec):
    from contextlib import ExitStack

    import concourse.bass as bass
    import concourse.mybir as mybir
    import concourse.tile as tile
    from concourse import bacc

    fp32 = mybir.dt.float32
    fp16 = mybir.dt.float16
    Exp = mybir.ActivationFunctionType.Exp
    mult = mybir.AluOpType.mult
    add = mybir.AluOpType.add

    NK = sum(nk for nk, _ in spec)
    NQ = sum(sum(ws) // P for _, ws in spec)
    TK = NK * P
    WQ = NQ * P

    nc = bacc.Bacc("TRN2", target_bir_lowering=False, debug=False)

    xt_d = nc.dram_tensor("xt", [P, KC, TK], fp16, kind="ExternalInput").ap()
    kbias_d = nc.dram_tensor("kbias", [P, NK], fp32, kind="ExternalInput").ap()
    qmask_d = nc.dram_tensor("qmask", [P, NQ], fp32, kind="ExternalInput").ap()
    w_d = {
        name: nc.dram_tensor(name, [P, KC, D], fp16, kind="ExternalInput").ap()
        for name in ("wq", "wk", "wv", "wo")
    }
    bo_d = nc.dram_tensor("bo", [D], fp32, kind="ExternalInput").ap()
    out_d = nc.dram_tensor("out", [WQ, D], fp32, kind="ExternalOutput").ap()

    with ExitStack() as ctx:
        tc = ctx.enter_context(tile.TileContext(nc))
        singles = ctx.enter_context(tc.tile_pool(name="singles", bufs=1))
        big = ctx.enter_context(tc.tile_pool(name="big", bufs=1))
        epool = ctx.enter_context(tc.tile_pool(name="epool", bufs=3))
        opool = ctx.enter_context(tc.tile_pool(name="opool", bufs=4))
        mmps = ctx.enter_context(tc.tile_pool(name="mmps", bufs=2, space="PSUM"))
        scps = ctx.enter_context(tc.tile_pool(name="scps", bufs=2, space="PSUM"))
        accps = ctx.enter_context(tc.tile_pool(name="accps", bufs=1, space="PSUM"))

        ones64 = singles.tile([P, DH], fp16)
        nc.vector.memset(ones64, 1.0)

        w_sb = {}
        for name in ("wk", "wv", "wq", "wo"):
            w_sb[name] = singles.tile([P, KC, D], fp16, name=f"w_{name}")
            nc.sync.dma_start(out=w_sb[name], in_=w_d[name])

        xT = big.tile([P, KC, TK], fp16, name="xT")
        # chunked x loads so compute can start early
        for ts in range(0, TK, 512):
            w = min(512, TK - ts)
            nc.sync.dma_start(
                out=xT[:, :, ts : ts + w], in_=xt_d[:, :, ts : ts + w]
            )

        kbias_sb = singles.tile([P, NK], fp32)
        nc.sync.dma_start(out=kbias_sb, in_=kbias_d)
        qmask_sb = singles.tile([P, NQ], fp32)
        nc.sync.dma_start(out=qmask_sb, in_=qmask_d)
        bo_rep = singles.tile([P, D], fp32)
        bo_bcast = bass.AP(tensor=bo_d.tensor, offset=bo_d.offset, ap=[[0, P], [1, D]])
        nc.gpsimd.dma_start(out=bo_rep, in_=bo_bcast)

        KT = big.tile([P, KC, TK], fp16, name="KT")
        QT = big.tile([P, KC, WQ], fp16, name="QT")
        outT = big.tile([P, KC, WQ], fp16, name="outT")
        V = big.tile([P, NK, D], fp16, name="V")

        # ---------- unit generators (each returns a list of closures) ----
        def kt_units(hp, ts, w):
            """KT[:, hp, ts:ts+w] = Wk[:,:,hp-block]^T @ xT[:,:,ts:ts+w]"""
            ps_box = []

            def mk_mm(kc):
                def emit():
                    if not ps_box:
                        ps_box.append(mmps.tile([P, 512], fp32, name="mm_ps", tag="mm"))
                    nc.tensor.matmul(
                        ps_box[0][:, :w],
                        w_sb["wk"][:, kc, hp * P : (hp + 1) * P],
                        xT[:, kc, ts : ts + w],
                        start=(kc == 0),
                        stop=(kc == KC - 1),
                    )
                return emit

            def fin():
                nc.vector.tensor_copy(
                    out=KT[:, hp, ts : ts + w], in_=ps_box[0][:, :w]
                )

            return [mk_mm(kc) for kc in range(KC)] + [fin]

        def qt_units(hp, qs, xs, w):
            """QT[:, hp, qs:qs+w] from xT[:, :, xs:xs+w]"""
            ps_box = []

            def mk_mm(kc):
                def emit():
                    if not ps_box:
                        ps_box.append(mmps.tile([P, 512], fp32, name="mm_ps", tag="mm"))
                    nc.tensor.matmul(
                        ps_box[0][:, :w],
                        w_sb["wq"][:, kc, hp * P : (hp + 1) * P],
                        xT[:, kc, xs : xs + w],
                        start=(kc == 0),
                        stop=(kc == KC - 1),
                    )
                return emit

            def fin():
                nc.vector.tensor_copy(
                    out=QT[:, hp, qs : qs + w], in_=ps_box[0][:, :w]
                )

            return [mk_mm(kc) for kc in range(KC)] + [fin]

        def v_units(kt):
            ps_box = []

            def mk_mm(kc):
                def emit():
                    if not ps_box:
                        ps_box.append(mmps.tile([P, 512], fp32, name="mm_ps", tag="mm"))
                    nc.tensor.matmul(
                        ps_box[0],
                        xT[:, kc, kt * P : (kt + 1) * P],
                        w_sb["wv"][:, kc, :],
                        start=(kc == 0),
                        stop=(kc == KC - 1),
                    )
                return emit

            def fin():
                nc.vector.tensor_copy(out=V[:, kt, :], in_=ps_box[0])

            return [mk_mm(kc) for kc in range(KC)] + [fin]

        def outproj_units(qt):
            """out rows [qt*P, (qt+1)*P) from outT[:, :, qt*P:...]"""
            ps_box = []

            def mk_mm(hc):
                def emit():
                    if not ps_box:
                        ps_box.append(mmps.tile([P, 512], fp32, name="mm_ps", tag="mm"))
                    nc.tensor.matmul(
                        ps_box[0],
                        outT[:, hc, qt * P : (qt + 1) * P],
                        w_sb["wo"][:, hc, :],
                        start=(hc == 0),
                        stop=(hc == KC - 1),
                    )
                return emit

            def fin():
                fout = opool.tile([P, D], fp32, tag="fout")
                nc.vector.tensor_tensor(fout, ps_box[0], bo_rep, add)
                nc.vector.tensor_scalar_mul(fout, fout, qmask_sb[:, qt : qt + 1])
                nc.sync.dma_start(out=out_d[qt * P : (qt + 1) * P, :], in_=fout)

            return [mk_mm(hc) for hc in range(KC)] + [fin]

        def attn_chunk(koff, nk, hp, qs, w, filler, iters_left):
            """Attention for one (group, head-pair, q-chunk)."""
            o_ps = accps.tile([P, 512], fp32, name="o_ps", tag="o_ps")
            d_ps = accps.tile([P, 512], fp32, name="d_ps", tag="d_ps")

            def emit_scores_exp(kt):
                s_pair = scps.tile([P, 1024], fp32, name="s_pair", tag="s_pair")
                nc.tensor.matmul(
                    s_pair[:, 0:w],
                    KT[0:DH, hp, (koff + kt) * P : (koff + kt + 1) * P],
                    QT[0:DH, hp, qs : qs + w],
                    start=True, stop=True, tile_position=(0, 0),
                )
                nc.tensor.matmul(
                    s_pair[:, 512 : 512 + w],
                    KT[DH:P, hp, (koff + kt) * P : (koff + kt + 1) * P],
                    QT[DH:P, hp, qs : qs + w],
                    start=True, stop=True, tile_position=(DH, 0),
                )
                e_pair = epool.tile([P, 2, 512], fp16, name="e_pair", tag="e_pair")
                nc.scalar.activation(
                    e_pair[:, :, :w],
                    s_pair.rearrange("p (h q) -> p h q", h=2)[:, :, :w],
                    Exp, bias=kbias_sb[:, koff + kt : koff + kt + 1],
                    scale=DH ** -0.5,
                )
                return e_pair

            def emit_pv(kt, e_pair):
                first, last = kt == 0, kt == nk - 1
                nc.tensor.matmul(
                    o_ps[0:DH, :w], V[:, koff + kt, hp * P : hp * P + DH],
                    e_pair[:, 0, :w], start=first, stop=last,
                    tile_position=(0, 0), skip_group_check=True,
                )
                nc.tensor.matmul(
                    o_ps[DH:P, :w], V[:, koff + kt, hp * P + DH : (hp + 1) * P],
                    e_pair[:, 1, :w], start=first, stop=last,
                    tile_position=(0, DH), skip_group_check=True,
                )
                nc.tensor.matmul(
                    d_ps[0:DH, :w], ones64, e_pair[:, 0, :w],
                    start=first, stop=last,
                    tile_position=(0, 0), skip_group_check=True,
                )
                nc.tensor.matmul(
                    d_ps[DH:P, :w], ones64, e_pair[:, 1, :w],
                    start=first, stop=last,
                    tile_position=(0, DH), skip_group_check=True,
                )

            pending = None
            for kt in range(nk):
                e_pair = emit_scores_exp(kt)
                if pending is not None:
                    emit_pv(*pending)
                pending = (kt, e_pair)
                if filler and iters_left[0] > 0:
                    k = -(-len(filler) // iters_left[0])
                    for _ in range(min(k, len(filler))):
                        filler.pop(0)()
                iters_left[0] -= 1
            emit_pv(*pending)
            rrep = epool.tile([P, 512], fp32, tag="rrep", bufs=2)
            nc.vector.reciprocal_approx_fast(out=rrep[:, :w], in_=d_ps[:, :w])
            nc.vector.tensor_tensor(
                outT[:, hp, qs : qs + w], o_ps[:, :w], rrep[:, :w], mult
            )

        # ---------- choreography ------------------------------------------
        # group geometry
        koffs, qoffs = [], []
        ko = qo = 0
        for nk, ws in spec:
            koffs.append(ko)
            qoffs.append(qo)
            ko += nk
            qo += sum(ws) // P

        # Per (group, chunk, hp) attention blocks, in execution order:
        # group-major, chunk-major, hp-minor (so a finished chunk's
        # outproj can ride later blocks).
        blocks = []
        for g, (nk, ws) in enumerate(spec):
            cq = 0
            for ci, w in enumerate(ws):
                for hp in range(KC):
                    blocks.append((g, ci, cq, w, hp))
                cq += w

        # Pre-work units per block: everything needed *before* that block
        # that isn't needed by an earlier block rides as filler of earlier
        # blocks; the first block's needs are emitted up front.
        # need(g, hp): KT[hp] over group g + V over group g (hp==0 only)
        #              + QT[hp] for each chunk of g
        kt_done = set()
        v_done = set()
        qt_done = set()

        def need_units(g, hp):
            nk, ws = spec[g]
            units = []
            if (g, hp) not in kt_done:
                kt_done.add((g, hp))
                ts0 = koffs[g] * P
                tk = nk * P
                for ts in range(0, tk, 512):
                    units += kt_units(hp, ts0 + ts, min(512, tk - ts))
            if g not in v_done:
                v_done.add(g)
                for kt in range(nk):
                    units += v_units(koffs[g] + kt)
            cq = 0
            for ci, w in enumerate(ws):
                if (g, ci, hp) not in qt_done:
                    qt_done.add((g, ci, hp))
                    units += qt_units(
                        hp, qoffs[g] * P + cq, koffs[g] * P + cq, w
                    )
                cq += w
            return units

        # assemble fillers: for block j, fillers are need_units of block
        # j+1 plus outproj of any chunk fully finished by block j-1.
        first_g, first_ci, first_cq, first_w, first_hp = blocks[0]
        for u in need_units(first_g, first_hp):
            u()

        pending_out = []  # outproj unit lists not yet scheduled
        filler: list = []
        for j, (g, ci, cq, w, hp) in enumerate(blocks):
            # queue pre-work for the NEXT block as filler of this one
            if j + 1 < len(blocks):
                ng, nci, ncq, nw, nhp = blocks[j + 1]
                filler.extend(need_units(ng, nhp))
            # outproj of a chunk finished in an EARLIER block may ride now
            if j + 1 < len(blocks) and pending_out:
                filler.extend(pending_out.pop(0))
            # queue outproj of the current chunk once its last hp completes
            if hp == KC - 1:
                nq_tiles = w // P
                q0 = qoffs[g] + cq // P
                pending_out.append([
                    u for t in range(nq_tiles) for u in outproj_units(q0 + t)
                ])
            nk = spec[g][0]
            iters_left = [nk]
            attn_chunk(koffs[g], nk, hp, qoffs[g] * P + cq, w, filler, iters_left)
            while filler and j + 1 == len(blocks):
                filler.pop(0)()
        # drain remaining fillers and outproj
        while filler:
            filler.pop(0)()
        for units in pending_out:
            for u in units:
                u()

    nc.compile()
    return nc


def _get_program(spec):
    if spec not in _BUILD_CACHE:
        _BUILD_CACHE[spec] = _build_bass(spec)
    return _BUILD_CACHE[spec]


# --------------------------------------------------------------------------
# launcher: one executable per core, dispatched async
# --------------------------------------------------------------------------

def _make_fn(nc, core_id, device):
    import jax
    import concourse.mybir as mybir
    from concourse import bass2jax

    in_names = []
    out_names = []
    out_avals = []
    out_shapes = []
    for alloc in nc.m.functions[0].allocations:
        if not isinstance(alloc, mybir.MemoryLocationSet):
            continue
        name = alloc.memorylocations[0].name
        if alloc.kind == "ExternalInput":
            in_names.append(name)
        elif alloc.kind == "ExternalOutput":
            out_names.append(name)
            shape = tuple(alloc.tensor_shape)
            dtype = mybir.dt.np(alloc.dtype)
            out_avals.append(jax.core.ShapedArray(shape, dtype))
            out_shapes.append((shape, dtype))
    all_names = in_names + out_names

    def _body(*args):
        outs = bass2jax._bass_exec_p.bind(
            *args,
            out_avals=tuple(out_avals),
            in_names=tuple(all_names),
            out_names=tuple(out_names),
            lowering_input_output_aliases=(),
            sim_require_finite=True,
            sim_require_nnan=True,
            nc=nc,
        )
        return tuple(outs)

    return jax.jit(_body), in_names, out_names, out_shapes


def _launch(per_core):
    """per_core: list of (spec, in_map). Returns list of out dicts."""
    import jax

    from concourse.bass2jax import install_neuronx_cc_hook

    install_neuronx_cc_hook()
    devices = jax.devices()

    handles = []
    for c, (spec, in_map) in enumerate(per_core):
        nc = _get_program(spec)
        key = (spec, c)
        if key not in _FN_CACHE:
            _FN_CACHE[key] = _make_fn(nc, c, devices[c])
        fn, in_names, out_names, out_shapes = _FN_CACHE[key]
        args = []
        for name in in_names:
            if name == "partition_id":
                a = np.array([[c]], dtype=np.uint32)
            else:
                a = in_map[name]
            args.append(jax.device_put(a, devices[c]))
        for shape, dtype in out_shapes:
            args.append(jax.device_put(np.zeros(shape, dtype), devices[c]))
        handles.append((fn, args, out_names))

    results = []
    outs = [fn(*args) for fn, args, _ in handles]
    for (fn, args, out_names), out in zip(handles, outs):
        jax.block_until_ready(out)
        results.append(
            {name: np.asarray(out[i]) for i, name in enumerate(out_names)}
        )
    return results


# --------------------------------------------------------------------------
# host-side data packing + entry point
# --------------------------------------------------------------------------

def kernel(x, seq_lens, Wq, Wk, Wv, Wo, bo) -> np.ndarray:
    x = np.asarray(x, dtype=np.float32)
    seq_lens_np = np.asarray(seq_lens, dtype=np.int32)
    Wq = np.asarray(Wq, dtype=np.float32)
    Wk = np.asarray(Wk, dtype=np.float32)
    Wv = np.asarray(Wv, dtype=np.float32)
    Wo = np.asarray(Wo, dtype=np.float32)
    bo = np.ascontiguousarray(np.asarray(bo, dtype=np.float32))

    assign = _plan(seq_lens_np)

    # weights: [D, D] -> [P, KC, D] fp16  (feature index kc*128+p)
    def wprep(W):
        return np.ascontiguousarray(
            W.reshape(KC, P, D).transpose(1, 0, 2).astype(np.float16)
        )

    w16 = {
        "wq": wprep(Wq), "wk": wprep(Wk), "wv": wprep(Wv), "wo": wprep(Wo)
    }

    # x pre-transposed per sequence: xt_seq[s] = [P, KC, ceil128] fp16
    n_tiles = [max(1, -(-int(L) // P)) for L in seq_lens_np]
    xt_seq = {}
    for i in range(B):
        nk = n_tiles[i]
        xs = x[i, : nk * P, :].astype(np.float16)  # [nk*P, D]
        # -> [P(feat within chunk), KC, tokens]
        xt_seq[i] = np.ascontiguousarray(
            xs.reshape(nk * P, KC, P).transpose(2, 1, 0)
        )

    pos_in_tile = np.arange(P, dtype=np.int32)

    per_core = []
    scatter = []  # per core: list of (seq, tile) in q order
    for c in range(N_CORES):
        groups = assign[c]
        spec = _spec_of(groups)
        NK = sum(nk for nk, _ in spec)
        NQ = sum(sum(ws) // P for _, ws in spec)
        xt = np.zeros((P, KC, NK * P), dtype=np.float16)
        kbias = np.full((P, NK), -60.0, dtype=np.float32)
        qmask = np.zeros((P, NQ), dtype=np.float32)
        qlist = []
        ko = qo = 0
        for (seq, nk, qts) in groups:
            L = int(seq_lens_np[seq])
            # permute tiles: owned q-tiles first, then the rest
            order = list(qts) + [t for t in range(nk) if t not in qts]
            for slot, t in enumerate(order):
                xt[:, :, (ko + slot) * P : (ko + slot + 1) * P] = xt_seq[seq][
                    :, :, t * P : (t + 1) * P
                ]
                valid = (t * P + pos_in_tile) < L
                kbias[:, ko + slot] = np.where(valid, 0.0, -60.0)
            for qi, t in enumerate(qts):
                valid = (t * P + pos_in_tile) < L
                qmask[:, qo + qi] = valid.astype(np.float32)
                qlist.append((seq, t))
            ko += nk
            qo += len(qts)
        in_map = {
            "xt": xt,
            "kbias": kbias,
            "qmask": qmask,
            "bo": bo,
            **w16,
        }
        per_core.append((spec, in_map))
        scatter.append(qlist)

    trace = bool(int(os.environ.get("KERNEL_TRACE", "0")))
    if trace:
        results, prof_dir = _launch_traced(per_core)
        kernel.last_profile_dir = prof_dir
    else:
        results = _launch(per_core)

    out = np.zeros((B, S, D), dtype=np.float32)
    for c in range(N_CORES):
        o = results[c]["out"]
        for qi, (seq, t) in enumerate(scatter[c]):
            out[seq, t * P : (t + 1) * P, :] = o[qi * P : (qi + 1) * P, :]
    return out


def _launch_traced(per_core):
    """Wrap the launch in an NTFF profile session (dev loop only)."""
    import tempfile

    from antenv.axon_hooks import get_axon_ntff_profile_hook

    hook = get_axon_ntff_profile_hook()
    prof_dir = tempfile.mkdtemp(prefix="kprof_")
    if hook is None:
        return _launch(per_core), None
    with hook(prof_dir, list(range(N_CORES))):
        results = _launch(per_core)
    return results, prof_dir


# revision 12
# speedup vs baseline: 1.4606x; 1.0756x over previous
"""Ragged-sequence multi-head attention (B=16, S=1024, D=512, H=8, DH=64)
for 8 Trainium2 NeuronCores.

Strategy: per-core specialized Bass programs over a balanced ragged job
assignment. Each sequence is rounded up to 128-token tiles; long
sequences are split by query-tile range across cores (each split re-hosts
the full K/V of its parent sequence). The host:
  - pre-transposes x into feature-major fp16 tiles (no PE transposes),
  - pre-converts the four projection weights to fp16 in [P, KC, D] layout,
  - packs each core's sequences into a flat tile arena with per-tile
    key-validity biases and query masks,
  - compiles one Bass program per distinct core *structure* (programs are
    shared between cores whose job shapes match; data differs via inputs),
  - dispatches all 8 single-core executables asynchronously (they overlap
    on the 8 NeuronCores) and scatters the per-tile outputs back.

Per-core pipeline (all matmuls fp16 operands, fp32 PSUM):
  KT/QT = W^T @ xT (feature-major), V in [tile, d] layout, then per
  head-pair/q-chunk/k-tile: scoresT = K^T q (row-packed pair),
  expT = exp(0.125*scoresT + kbias) on ACT, outT += V^T expT and
  denom += 1^T expT (col-packed pairs), normalize by reciprocal,
  out = outT^T @ Wo + bo, query-masked, DMA out.
"""

import math
import os

import numpy as np

B, S, D = 16, 1024, 512
H, DH = 8, 64
N_CORES = 8
P = 128
KC = D // P  # 4 contraction chunks of 128

_BUILD_CACHE: dict = {}
_FN_CACHE: dict = {}


# --------------------------------------------------------------------------
# planning: split sequences into blocks, balance blocks across cores
# --------------------------------------------------------------------------

def _chunks_of(m_tiles: int) -> tuple:
    """Split m q-tiles into chunk widths (tokens), biggest first, <=512."""
    out = []
    left = m_tiles * P
    while left > 0:
        w = min(512, left)
        out.append(w)
        left -= w
    return tuple(out)


def _act_cost(nk, m):
    """ACT-queue ns for attention of a (nk-key-tile, m-q-tile) block."""
    t = 0.0
    for w in _chunks_of(m):
        t += nk * 4 * ((2 * w + 352) / 1.2 + 270.0)
    return t


def _pe_cost(nk, m, host):
    """PE ns: attention slots (scores 2 passes + PV 1) + KV hosting +
    Q/O projections, all fp16."""
    t = 0.0
    for w in _chunks_of(m):
        t += nk * 4 * (3 * w / 2.4 + 150.0)
    if host:
        t += 1728.0 * nk
    t += 1728.0 * m
    return t


def _plan(seq_lens: np.ndarray):
    """Returns per-core list of groups (seq, nk, q_tiles list)."""
    n = [max(1, -(-int(L) // P)) for L in seq_lens]
    blocks = [(i, nk, list(range(nk))) for i, nk in enumerate(n)]
    blocks.sort(key=lambda b: -max(_act_cost(b[1], len(b[2])),
                                   _pe_cost(b[1], len(b[2]), True)))
    cores = [{"pe": 0.0, "act": 0.0, "groups": []} for _ in range(N_CORES)]

    def place(c, seq, nk, qts):
        c["pe"] += _pe_cost(nk, len(qts), True)
        c["act"] += _act_cost(nk, len(qts))
        c["groups"].append((seq, nk, qts))

    def score_after(c, nk, m):
        pe = c["pe"] + _pe_cost(nk, m, True)
        act = c["act"] + _act_cost(nk, m)
        return max(pe, act)

    def core_score(c):
        return max(c["pe"], c["act"])

    avg = max(
        sum(_act_cost(b[1], len(b[2])) for b in blocks) / N_CORES,
        sum(_pe_cost(b[1], len(b[2]), True) for b in blocks) / N_CORES,
    )
    for seq, nk, qts in blocks:
        m = len(qts)
        # option A: whole block on the best single core
        best_a, sc_a = None, None
        for c in cores:
            s = score_after(c, nk, m)
            if sc_a is None or s < sc_a:
                best_a, sc_a = c, s
        if m >= 5 and sc_a > 1.08 * avg:
            # split at a 4-tile boundary across the two least-loaded cores
            m1 = min(4, m - 1)
            order = sorted(cores, key=core_score)
            c1, c2 = order[0], order[1]
            place(c1, seq, nk, qts[:m1])
            place(c2, seq, nk, qts[m1:])
        else:
            place(best_a, seq, nk, qts)

    # local search: move single blocks off the worst core while it helps
    def unplace(c, g):
        seq, nk, qts = g
        c["pe"] -= _pe_cost(nk, len(qts), True)
        c["act"] -= _act_cost(nk, len(qts))
        c["groups"].remove(g)

    for _ in range(32):
        worst = max(cores, key=core_score)
        best_move, best_gain = None, 0.0
        cur = core_score(worst)
        for g in worst["groups"]:
            seq, nk, qts = g
            for c in cores:
                if c is worst:
                    continue
                new_dst = score_after(c, nk, len(qts))
                unplace(worst, g)
                new_src = core_score(worst)
                place(worst, seq, nk, qts)
                gain = cur - max(new_dst, new_src)
                if gain > best_gain + 1.0:
                    best_move, best_gain = (g, c), gain
        if best_move is None:
            break
        g, c = best_move
        unplace(worst, g)
        place(c, g[0], g[1], g[2])
    return [c["groups"] for c in cores]


# --------------------------------------------------------------------------
# program generator, parameterized by core structure
# --------------------------------------------------------------------------

def _spec_of(groups):
    """Structural spec: tuple of (nk, chunk widths). Data-independent."""
    return tuple((nk, _chunks_of(len(qts))) for _, nk, qts in groups)


def _build_bass(spec):
    from contextlib import ExitStack

    import concourse.bass as bass
    import concourse.mybir as mybir
    import concourse.tile as tile
    from concourse import bacc

    fp32 = mybir.dt.float32
    fp16 = mybir.dt.float16
    fp8 = mybir.dt.float8e4
    DR = mybir.MatmulPerfMode.DoubleRow
    Exp = mybir.ActivationFunctionType.Exp
    mult = mybir.AluOpType.mult
    add = mybir.AluOpType.add

    NK = sum(nk for nk, _ in spec)
    NQ = sum(sum(ws) // P for _, ws in spec)
    TK = NK * P
    WQ = NQ * P

    nc = bacc.Bacc("TRN2", target_bir_lowering=False, debug=False)

    xt_d = nc.dram_tensor("xt", [P, KC, TK], fp16, kind="ExternalInput").ap()
    kbias_d = nc.dram_tensor("kbias", [P, NK], fp32, kind="ExternalInput").ap()
    qmask_d = nc.dram_tensor("qmask", [P, NQ], fp32, kind="ExternalInput").ap()
    w_d = {
        name: nc.dram_tensor(name, [P, KC, D], fp16, kind="ExternalInput").ap()
        for name in ("wq", "wk", "wv", "wo")
    }
    bo_d = nc.dram_tensor("bo", [D], fp32, kind="ExternalInput").ap()
    out_d = nc.dram_tensor("out", [WQ, D], fp32, kind="ExternalOutput").ap()

    with ExitStack() as ctx:
        tc = ctx.enter_context(tile.TileContext(nc))
        singles = ctx.enter_context(tc.tile_pool(name="singles", bufs=1))
        big = ctx.enter_context(tc.tile_pool(name="big", bufs=1))
        epool = ctx.enter_context(tc.tile_pool(name="epool", bufs=3))
        dpool = ctx.enter_context(tc.tile_pool(name="dpool", bufs=2))
        opool = ctx.enter_context(tc.tile_pool(name="opool", bufs=4))
        mmps = ctx.enter_context(tc.tile_pool(name="mmps", bufs=2, space="PSUM"))
        scps = ctx.enter_context(tc.tile_pool(name="scps", bufs=2, space="PSUM"))
        accps = ctx.enter_context(tc.tile_pool(name="accps", bufs=1, space="PSUM"))

        ones64 = singles.tile([P, DH], fp16)
        nc.vector.memset(ones64, 1.0)

        # DMA order matters: first-needed first (wk, x of group 0, wq,
        # then the rest). The sync DMA queue drains in issue order.
        w_sb = {
            name: singles.tile([P, KC, D], fp16, name=f"w_{name}")
            for name in ("wk", "wv", "wq", "wo")
        }
        xT = big.tile([P, KC, TK], fp16, name="xT")
        g0k = spec[0][0] * P  # group-0 token extent

        nc.sync.dma_start(out=w_sb["wk"], in_=w_d["wk"])
        for ts in range(0, g0k, 1024):
            w = min(1024, g0k - ts)
            nc.sync.dma_start(
                out=xT[:, :, ts : ts + w], in_=xt_d[:, :, ts : ts + w]
            )
        nc.sync.dma_start(out=w_sb["wq"], in_=w_d["wq"])
        nc.sync.dma_start(out=w_sb["wv"], in_=w_d["wv"])
        kbias_sb = singles.tile([P, NK], fp32)
        nc.sync.dma_start(out=kbias_sb, in_=kbias_d)
        qmask_sb = singles.tile([P, NQ], fp32)
        nc.sync.dma_start(out=qmask_sb, in_=qmask_d)
        for ts in range(g0k, TK, 1024):
            w = min(1024, TK - ts)
            nc.sync.dma_start(
                out=xT[:, :, ts : ts + w], in_=xt_d[:, :, ts : ts + w]
            )
        nc.sync.dma_start(out=w_sb["wo"], in_=w_d["wo"])
        bo_rep = singles.tile([P, D], fp32)
        bo_bcast = bass.AP(tensor=bo_d.tensor, offset=bo_d.offset, ap=[[0, P], [1, D]])
        nc.gpsimd.dma_start(out=bo_rep, in_=bo_bcast)

        KT = big.tile([P, KC, TK], fp16, name="KT")
        QT = big.tile([P, KC, WQ], fp16, name="QT")
        outT = big.tile([P, KC, WQ], fp16, name="outT")
        V = big.tile([P, NK, D], fp16, name="V")

        # ---------- unit generators (each returns a list of closures) ----
        def kt_units(hp, ts, w):
            """KT[:, hp, ts:ts+w] = Wk[:,:,hp-block]^T @ xT[:,:,ts:ts+w]"""
            ps_box = []

            def mk_mm(kc):
                def emit():
                    if not ps_box:
                        ps_box.append(mmps.tile([P, 512], fp32, name="mm_ps", tag="mm"))
                    nc.tensor.matmul(
                        ps_box[0][:, :w],
                        w_sb["wk"][:, kc, hp * P : (hp + 1) * P],
                        xT[:, kc, ts : ts + w],
                        start=(kc == 0),
                        stop=(kc == KC - 1),
                    )
                return emit

            def fin():
                nc.vector.tensor_copy(
                    out=KT[:, hp, ts : ts + w], in_=ps_box[0][:, :w]
                )

            return [mk_mm(kc) for kc in range(KC)] + [fin]

        def qt_units(hp, qs, xs, w):
            """QT[:, hp, qs:qs+w] from xT[:, :, xs:xs+w]"""
            ps_box = []

            def mk_mm(kc):
                def emit():
                    if not ps_box:
                        ps_box.append(mmps.tile([P, 512], fp32, name="mm_ps", tag="mm"))
                    nc.tensor.matmul(
                        ps_box[0][:, :w],
                        w_sb["wq"][:, kc, hp * P : (hp + 1) * P],
                        xT[:, kc, xs : xs + w],
                        start=(kc == 0),
                        stop=(kc == KC - 1),
                    )
                return emit

            def fin():
                nc.vector.tensor_copy(
                    out=QT[:, hp, qs : qs + w], in_=ps_box[0][:, :w]
                )

            return [mk_mm(kc) for kc in range(KC)] + [fin]

        def v_units(kt):
            ps_box = []

            def mk_mm(kc):
                def emit():
                    if not ps_box:
                        ps_box.append(mmps.tile([P, 512], fp32, name="mm_ps", tag="mm"))
                    nc.tensor.matmul(
                        ps_box[0],
                        xT[:, kc, kt * P : (kt + 1) * P],
                        w_sb["wv"][:, kc, :],
                        start=(kc == 0),
                        stop=(kc == KC - 1),
                    )
                return emit

            def fin():
                nc.vector.tensor_copy(out=V[:, kt, :], in_=ps_box[0])

            return [mk_mm(kc) for kc in range(KC)] + [fin]

        def outproj_units(qt):
            """out rows [qt*P, (qt+1)*P) from outT[:, :, qt*P:...]"""
            ps_box = []

            def mk_mm(hc):
                def emit():
                    if not ps_box:
                        ps_box.append(mmps.tile([P, 512], fp32, name="mm_ps", tag="mm"))
                    nc.tensor.matmul(
                        ps_box[0],
                        outT[:, hc, qt * P : (qt + 1) * P],
                        w_sb["wo"][:, hc, :],
                        start=(hc == 0),
                        stop=(hc == KC - 1),
                    )
                return emit

            def fin():
                fout = opool.tile([P, D], fp32, tag="fout")
                nc.vector.tensor_tensor(fout, ps_box[0], bo_rep, add)
                nc.vector.tensor_scalar_mul(fout, fout, qmask_sb[:, qt : qt + 1])
                nc.sync.dma_start(out=out_d[qt * P : (qt + 1) * P, :], in_=fout)

            return [mk_mm(hc) for hc in range(KC)] + [fin]

        def attn_chunk(koff, nk, hp, qs, w, filler, iters_left):
            """Attention for one (group, head-pair, q-chunk). The softmax
            denominator accumulates on the DVE (e-tile adds) and is
            reduced across partitions by one ones-matmul per chunk."""
            o_ps = accps.tile([P, 512], fp32, name="o_ps", tag="o_ps")
            d_ps = accps.tile([P, 512], fp32, name="d_ps", tag="d_ps")
            d_acc = dpool.tile([P, 2, 512], fp16, name="d_acc", tag="d_acc")

            def emit_scores_exp(kt):
                s_pair = scps.tile([P, 1024], fp32, name="s_pair", tag="s_pair")
                nc.tensor.matmul(
                    s_pair[:, 0:w],
                    KT[0:DH, hp, (koff + kt) * P : (koff + kt + 1) * P],
                    QT[0:DH, hp, qs : qs + w],
                    start=True, stop=True, tile_position=(0, 0),
                )
                nc.tensor.matmul(
                    s_pair[:, 512 : 512 + w],
                    KT[DH:P, hp, (koff + kt) * P : (koff + kt + 1) * P],
                    QT[DH:P, hp, qs : qs + w],
                    start=True, stop=True, tile_position=(DH, 0),
                )
                e_pair = epool.tile([P, 2, 512], fp16, name="e_pair", tag="e_pair")
                nc.scalar.activation(
                    e_pair[:, :, :w],
                    s_pair.rearrange("p (h q) -> p h q", h=2)[:, :, :w],
                    Exp, bias=kbias_sb[:, koff + kt : koff + kt + 1],
                    scale=DH ** -0.5,
                )
                return e_pair

            def emit_pv(kt, e_pair):
                first, last = kt == 0, kt == nk - 1
                nc.tensor.matmul(
                    o_ps[0:DH, :w], V[:, koff + kt, hp * P : hp * P + DH],
                    e_pair[:, 0, :w], start=first, stop=last,
                    tile_position=(0, 0), skip_group_check=True,
                )
                nc.tensor.matmul(
                    o_ps[DH:P, :w], V[:, koff + kt, hp * P + DH : (hp + 1) * P],
                    e_pair[:, 1, :w], start=first, stop=last,
                    tile_position=(0, DH), skip_group_check=True,
                )
                if first:
                    nc.vector.tensor_copy(
                        out=d_acc[:, :, :w], in_=e_pair[:, :, :w]
                    )
                else:
                    nc.vector.tensor_tensor(
                        d_acc[:, :, :w], d_acc[:, :, :w], e_pair[:, :, :w], add
                    )

            pending = None
            for kt in range(nk):
                e_pair = emit_scores_exp(kt)
                if pending is not None:
                    emit_pv(*pending)
                pending = (kt, e_pair)
                if filler and iters_left[0] > 0:
                    k = -(-len(filler) // iters_left[0])
                    for _ in range(min(k, len(filler))):
                        filler.pop(0)()
                iters_left[0] -= 1
            emit_pv(*pending)
            nc.tensor.matmul(
                d_ps[0:DH, :w], ones64, d_acc[:, 0, :w],
                start=True, stop=True,
                tile_position=(0, 0), skip_group_check=True,
            )
            nc.tensor.matmul(
                d_ps[DH:P, :w], ones64, d_acc[:, 1, :w],
                start=True, stop=True,
                tile_position=(0, DH), skip_group_check=True,
            )
            rrep = epool.tile([P, 512], fp32, tag="rrep", bufs=2)
            nc.vector.reciprocal_approx_fast(out=rrep[:, :w], in_=d_ps[:, :w])
            nc.vector.tensor_tensor(
                outT[:, hp, qs : qs + w], o_ps[:, :w], rrep[:, :w], mult
            )

        # ---------- choreography ------------------------------------------
        # group geometry
        koffs, qoffs = [], []
        ko = qo = 0
        for nk, ws in spec:
            koffs.append(ko)
            qoffs.append(qo)
            ko += nk
            qo += sum(ws) // P

        # Per (group, chunk, hp) attention blocks, in execution order:
        # group-major, chunk-major, hp-minor (so a finished chunk's
        # outproj can ride later blocks).
        blocks = []
        for g, (nk, ws) in enumerate(spec):
            cq = 0
            for ci, w in enumerate(ws):
                for hp in range(KC):
                    blocks.append((g, ci, cq, w, hp))
                cq += w

        # Pre-work units per block: everything needed *before* that block
        # that isn't needed by an earlier block rides as filler of earlier
        # blocks; the first block's needs are emitted up front.
        # need(g, hp): KT[hp] over group g + V over group g (hp==0 only)
        #              + QT[hp] for each chunk of g
        kt_done = set()
        v_done = set()
        qt_done = set()

        def need_units(g, ci, hp, split_first=False):
            """(upfront, deferrable) units needed before block (g,ci,hp)."""
            nk, ws = spec[g]
            up, defer = [], []
            if (g, hp) not in kt_done:
                kt_done.add((g, hp))
                ts0 = koffs[g] * P
                tk = nk * P
                for ts in range(0, tk, 512):
                    up += kt_units(hp, ts0 + ts, min(512, tk - ts))
            if g not in v_done:
                v_done.add(g)
                vts = [v_units(koffs[g] + kt) for kt in range(nk)]
                cut = 2 if (split_first and nk > 2) else nk
                for vt in vts[:cut]:
                    up += vt
                for vt in vts[cut:]:
                    defer += vt
            if (g, ci, hp) not in qt_done:
                qt_done.add((g, ci, hp))
                cq = sum(ws[:ci])
                up += qt_units(hp, qoffs[g] * P + cq, koffs[g] * P + cq, ws[ci])
            return up, defer

        # assemble fillers: for block j, fillers are need_units of block
        # j+1 plus outproj of any chunk fully finished by block j-1.
        first_g, first_ci, first_cq, first_w, first_hp = blocks[0]
        filler: list = []
        up, defer = need_units(first_g, first_ci, first_hp, split_first=True)
        for u in up:
            u()
        filler.extend(defer)

        pending_out = []  # outproj unit lists not yet scheduled
        for j, (g, ci, cq, w, hp) in enumerate(blocks):
            # queue pre-work for the NEXT block as filler of this one
            if j + 1 < len(blocks):
                ng, nci, ncq, nw, nhp = blocks[j + 1]
                up, defer = need_units(ng, nci, nhp)
                filler.extend(up)
                filler.extend(defer)
            # outproj of a chunk finished in an EARLIER block may ride now
            if j + 1 < len(blocks) and pending_out:
                filler.extend(pending_out.pop(0))
            # queue outproj of the current chunk once its last hp completes
            if hp == KC - 1:
                nq_tiles = w // P
                q0 = qoffs[g] + cq // P
                pending_out.append([
                    u for t in range(nq_tiles) for u in outproj_units(q0 + t)
                ])
            nk = spec[g][0]
            iters_left = [nk]
            attn_chunk(koffs[g], nk, hp, qoffs[g] * P + cq, w, filler, iters_left)
            while filler and j + 1 == len(blocks):
                filler.pop(0)()
        # drain remaining fillers and outproj
        while filler:
            filler.pop(0)()
        for units in pending_out:
            for u in units:
                u()

    nc.compile()
    return nc


def _get_program(spec):
    if spec not in _BUILD_CACHE:
        _BUILD_CACHE[spec] = _build_bass(spec)
    return _BUILD_CACHE[spec]


# --------------------------------------------------------------------------
# launcher: one executable per core, dispatched async
# --------------------------------------------------------------------------

def _make_fn(nc, core_id, device):
    import jax
    import concourse.mybir as mybir
    from concourse import bass2jax

    in_names = []
    out_names = []
    out_avals = []
    out_shapes = []
    for alloc in nc.m.functions[0].allocations:
        if not isinstance(alloc, mybir.MemoryLocationSet):
            continue
        name = alloc.memorylocations[0].name
        if alloc.kind == "ExternalInput":
            in_names.append(name)
        elif alloc.kind == "ExternalOutput":
            out_names.append(name)
            shape = tuple(alloc.tensor_shape)
            dtype = mybir.dt.np(alloc.dtype)
            out_avals.append(jax.core.ShapedArray(shape, dtype))
            out_shapes.append((shape, dtype))
    all_names = in_names + out_names

    def _body(*args):
        outs = bass2jax._bass_exec_p.bind(
            *args,
            out_avals=tuple(out_avals),
            in_names=tuple(all_names),
            out_names=tuple(out_names),
            lowering_input_output_aliases=(),
            sim_require_finite=True,
            sim_require_nnan=True,
            nc=nc,
        )
        return tuple(outs)

    return jax.jit(_body), in_names, out_names, out_shapes


def _launch(per_core):
    """per_core: list of (spec, in_map). Returns list of out dicts."""
    import jax

    from concourse.bass2jax import install_neuronx_cc_hook

    install_neuronx_cc_hook()
    devices = jax.devices()

    handles = []
    for c, (spec, in_map) in enumerate(per_core):
        nc = _get_program(spec)
        key = (spec, c)
        if key not in _FN_CACHE:
            _FN_CACHE[key] = _make_fn(nc, c, devices[c])
        fn, in_names, out_names, out_shapes = _FN_CACHE[key]
        args = []
        for name in in_names:
            if name == "partition_id":
                a = np.array([[c]], dtype=np.uint32)
            else:
                a = in_map[name]
            args.append(jax.device_put(a, devices[c]))
        for shape, dtype in out_shapes:
            args.append(jax.device_put(np.zeros(shape, dtype), devices[c]))
        handles.append((fn, args, out_names))

    results = []
    outs = [fn(*args) for fn, args, _ in handles]
    for (fn, args, out_names), out in zip(handles, outs):
        jax.block_until_ready(out)
        results.append(
            {name: np.asarray(out[i]) for i, name in enumerate(out_names)}
        )
    return results


# --------------------------------------------------------------------------
# host-side data packing + entry point
# --------------------------------------------------------------------------

def kernel(x, seq_lens, Wq, Wk, Wv, Wo, bo) -> np.ndarray:
    x = np.asarray(x, dtype=np.float32)
    seq_lens_np = np.asarray(seq_lens, dtype=np.int32)
    Wq = np.asarray(Wq, dtype=np.float32)
    Wk = np.asarray(Wk, dtype=np.float32)
    Wv = np.asarray(Wv, dtype=np.float32)
    Wo = np.asarray(Wo, dtype=np.float32)
    bo = np.ascontiguousarray(np.asarray(bo, dtype=np.float32))

    assign = _plan(seq_lens_np)

    # weights: [D, D] -> [P, KC, D] fp16  (feature index kc*128+p)
    def wprep(W):
        return np.ascontiguousarray(
            W.reshape(KC, P, D).transpose(1, 0, 2).astype(np.float16)
        )

    w16 = {
        "wq": wprep(Wq), "wk": wprep(Wk), "wv": wprep(Wv), "wo": wprep(Wo)
    }

    # x pre-transposed per sequence: xt_seq[s] = [P, KC, ceil128] fp16
    n_tiles = [max(1, -(-int(L) // P)) for L in seq_lens_np]
    xt_seq = {}
    for i in range(B):
        nk = n_tiles[i]
        xs = x[i, : nk * P, :].astype(np.float16)  # [nk*P, D]
        # -> [P(feat within chunk), KC, tokens]
        xt_seq[i] = np.ascontiguousarray(
            xs.reshape(nk * P, KC, P).transpose(2, 1, 0)
        )

    pos_in_tile = np.arange(P, dtype=np.int32)

    per_core = []
    scatter = []  # per core: list of (seq, tile) in q order
    for c in range(N_CORES):
        groups = assign[c]
        spec = _spec_of(groups)
        NK = sum(nk for nk, _ in spec)
        NQ = sum(sum(ws) // P for _, ws in spec)
        xt = np.zeros((P, KC, NK * P), dtype=np.float16)
        kbias = np.full((P, NK), -60.0, dtype=np.float32)
        qmask = np.zeros((P, NQ), dtype=np.float32)
        qlist = []
        ko = qo = 0
        for (seq, nk, qts) in groups:
            L = int(seq_lens_np[seq])
            # permute tiles: owned q-tiles first, then the rest
            order = list(qts) + [t for t in range(nk) if t not in qts]
            for slot, t in enumerate(order):
                xt[:, :, (ko + slot) * P : (ko + slot + 1) * P] = xt_seq[seq][
                    :, :, t * P : (t + 1) * P
                ]
                valid = (t * P + pos_in_tile) < L
                kbias[:, ko + slot] = np.where(valid, 0.0, -60.0)
            for qi, t in enumerate(qts):
                valid = (t * P + pos_in_tile) < L
                qmask[:, qo + qi] = valid.astype(np.float32)
                qlist.append((seq, t))
            ko += nk
            qo += len(qts)
        in_map = {
            "xt": xt,
            "kbias": kbias,
            "qmask": qmask,
            "bo": bo,
            **w16,
        }
        per_core.append((spec, in_map))
        scatter.append(qlist)

    trace = bool(int(os.environ.get("KERNEL_TRACE", "0")))
    if trace:
        results, prof_dir = _launch_traced(per_core)
        kernel.last_profile_dir = prof_dir
    else:
        results = _launch(per_core)

    out = np.zeros((B, S, D), dtype=np.float32)
    for c in range(N_CORES):
        o = results[c]["out"]
        for qi, (seq, t) in enumerate(scatter[c]):
            out[seq, t * P : (t + 1) * P, :] = o[qi * P : (qi + 1) * P, :]
    return out


def _launch_traced(per_core):
    """Wrap the launch in an NTFF profile session (dev loop only)."""
    import tempfile

    from antenv.axon_hooks import get_axon_ntff_profile_hook

    hook = get_axon_ntff_profile_hook()
    prof_dir = tempfile.mkdtemp(prefix="kprof_")
    if hook is None:
        return _launch(per_core), None
    with hook(prof_dir, list(range(N_CORES))):
        results = _launch(per_core)
    return results, prof_dir
